# revision 5
# baseline (speedup 1.0000x reference)
"""ColBERT in-batch-negatives loss on 8 Trainium2 NeuronCores.

Sharding: batch (b) axis of query_embeddings split across the 8 cores
(16 rows each); every core receives the full positive_embeddings (the
"all-gather" is done at input-distribution time since kernel() takes the
full inputs anyway). Each core computes its [16, 128] score slab

    score[b, c] = sum_s max_d  q[b, s, :] . p[c, d, :]

via PE matmuls (bf16 inputs, fp32 PSUM) + DVE segmented max-reduce + a
ones-matmul for the sum over s, then the per-sample CE partial
    loss[b] = logsumexp_c(score[b, :] / T) - score[b, b] / T
on-device. The host sums the 8x16 per-sample losses and divides by 128
(the mean "all-reduce" at unshard time).

B=128, S=32, D_TOK=128, H=128, TEMPERATURE=0.02 are hardcoded per spec.
"""
import numpy as np

import concourse.mybir as mybir
from concourse import bacc
from concourse.tile import TileContext
from concourse.bass_utils import run_bass_kernel_spmd

F32 = mybir.dt.float32
BF16 = mybir.dt.bfloat16

B, S, D_TOK, H = 128, 32, 128, 128
TEMPERATURE = 0.02
N_CORES = 8
B_LOC = B // N_CORES            # 16 batch rows per core
N_BG = B_LOC // 4               # 4 b-groups of 4 rows (4*32 = 128 partitions)
CD = B * D_TOK                  # 16384 columns of p^T
CHUNK = 2048                    # psum tile free size (4 banks), 16 c's
N_CHUNK = CD // CHUNK           # 8 chunks

_cache = {}


def _build():
    """Build + compile the SPMD bass kernel (once per process)."""
    if "nc" in _cache:
        return _cache["nc"]

    nc = bacc.Bacc("TRN2", target_bir_lowering=False, debug=False,
                   num_devices=N_CORES)
    qt = nc.dram_tensor("qt", [H, B_LOC * S], BF16, kind="ExternalInput").ap()
    pt = nc.dram_tensor("pt", [H, CD], BF16, kind="ExternalInput").ap()
    ones16 = nc.dram_tensor("ones16", [H, 4 * B_LOC], F32,
                            kind="ExternalInput").ap()
    dmask = nc.dram_tensor("dmask", [B_LOC, B], F32, kind="ExternalInput").ap()
    loss_vec = nc.dram_tensor("loss_vec", [B_LOC, 1], F32,
                              kind="ExternalOutput").ap()

    with TileContext(nc) as tc:
        with tc.tile_pool(name="sbuf", bufs=1) as pool, \
             tc.tile_pool(name="psum", bufs=1, space="PSUM") as psum_pool:
            qt_t = pool.tile([H, B_LOC * S], BF16)
            ones_t = pool.tile([H, 4 * B_LOC], F32)
            dmask_t = pool.tile([B_LOC, B], F32)
            # separate tile per chunk so the first matmul only waits on the
            # first chunk's DMA, not the whole 4 MiB transfer
            pt_tiles = [pool.tile([H, CHUNK], BF16, name=f"ptc{_j}")
                        for _j in range(N_CHUNK)]
            with nc.named_scope("load"):
                nc.gpsimd.dma_start(qt_t[:], qt[:])
                nc.gpsimd.dma_start(ones_t[:], ones16[:])
                nc.gpsimd.dma_start(dmask_t[:], dmask[:])
                for j in range(N_CHUNK):
                    nc.gpsimd.dma_start(pt_tiles[j][:],
                                        pt[:, j * CHUNK:(j + 1) * CHUNK])

            pA = psum_pool.tile([128, CHUNK], F32, name="pA")
            pB = psum_pool.tile([128, CHUNK], F32, name="pB")
            ptiles = [pA, pB]

            # m_all[:, g*128 + c] = max_d late for b-group g, batch-col c
            m_all = pool.tile([128, 4 * B], F32)

            with nc.named_scope("mm_reduce"):
                for g in range(N_BG):
                    stat = qt_t[:, g * 128:(g + 1) * 128]
                    for j in range(N_CHUNK):
                        pt_tile = ptiles[(g * N_CHUNK + j) % 2]
                        for k in range(CHUNK // 512):
                            nc.tensor.matmul(
                                pt_tile[:, k * 512:(k + 1) * 512],
                                stat,
                                pt_tiles[j][:, k * 512:(k + 1) * 512],
                                start=True, stop=True)
                        # segmented max over d: [128, 16, 128] -> [128, 16]
                        nc.vector.tensor_reduce(
                            m_all[:, g * B + j * (CHUNK // D_TOK):
                                  g * B + (j + 1) * (CHUNK // D_TOK)],
                            pt_tile[:].rearrange("p (c d) -> p c d",
                                                 d=D_TOK),
                            axis=mybir.AxisListType.X,
                            op=mybir.AluOpType.max)

            # scores[b, c] = sum_s m_all: 4 accumulating ones-matmuls into
            # partitions 0..15 of pA's first bank
            s_psum = pA[0:B_LOC, 0:B]
            with nc.named_scope("tail"):
                for g in range(N_BG):
                    nc.tensor.matmul(
                        s_psum, ones_t[:, g * B_LOC:(g + 1) * B_LOC],
                        m_all[:, g * B:(g + 1) * B],
                        start=(g == 0), stop=(g == N_BG - 1))

                s_all = pool.tile([B_LOC, B], F32)
                nc.scalar.activation(s_all[:], s_psum,
                                     mybir.ActivationFunctionType.Copy,
                                     bias=0.0, scale=1.0 / TEMPERATURE)
                r = pool.tile([B_LOC, 1], F32)
                nc.vector.tensor_reduce(r[:], s_all[:],
                                        axis=mybir.AxisListType.X,
                                        op=mybir.AluOpType.max)
                negr = pool.tile([B_LOC, 1], F32)
                nc.vector.tensor_scalar_mul(negr[:], r[:], -1.0)
                e = pool.tile([B_LOC, B], F32)
                z = pool.tile([B_LOC, 1], F32)
                nc.scalar.activation(e[:], s_all[:],
                                     mybir.ActivationFunctionType.Exp,
                                     bias=negr[:], scale=1.0,
                                     accum_out=z[:])
                logz = pool.tile([B_LOC, 1], F32)
                nc.scalar.activation(logz[:], z[:],
                                     mybir.ActivationFunctionType.Ln)
                lse = pool.tile([B_LOC, 1], F32)
                nc.vector.tensor_tensor(lse[:], r[:], logz[:],
                                        op=mybir.AluOpType.add)
                junk = pool.tile([B_LOC, B], F32)
                diag = pool.tile([B_LOC, 1], F32)
                nc.vector.tensor_tensor(junk[:], s_all[:], dmask_t[:],
                                        op=mybir.AluOpType.mult)
                nc.vector.tensor_reduce(diag[:], junk[:],
                                        axis=mybir.AxisListType.X,
                                        op=mybir.AluOpType.add)
                lv = pool.tile([B_LOC, 1], F32)
                nc.vector.tensor_tensor(lv[:], lse[:], diag[:],
                                        op=mybir.AluOpType.subtract)
                nc.gpsimd.dma_start(loss_vec[:], lv[:])

    nc.compile()
    _cache["nc"] = nc
    return nc


def _host_inputs(query_embeddings, positive_embeddings):
    """Shard + lay out host-side inputs for the 8 cores."""
    import ml_dtypes
    q = np.ascontiguousarray(query_embeddings, dtype=np.float32)
    p = np.ascontiguousarray(positive_embeddings, dtype=np.float32)
    # qt_full[h, b*S + s] = q[b, s, h]
    qt_full = np.ascontiguousarray(
        q.transpose(2, 0, 1).reshape(H, B * S)).astype(ml_dtypes.bfloat16)
    # pt[h, c*D + d] = p[c, d, h]
    pt = np.ascontiguousarray(
        p.transpose(2, 0, 1).reshape(H, CD)).astype(ml_dtypes.bfloat16)

    ones16 = np.zeros((H, 4 * B_LOC), dtype=np.float32)
    for g in range(N_BG):
        for k in range(128):
            ones16[k, g * B_LOC + g * 4 + k // S] = 1.0

    in_maps = []
    for core in range(N_CORES):
        dmask = np.zeros((B_LOC, B), dtype=np.float32)
        for i in range(B_LOC):
            dmask[i, core * B_LOC + i] = 1.0
        in_maps.append({
            "qt": np.ascontiguousarray(
                qt_full[:, core * B_LOC * S:(core + 1) * B_LOC * S]),
            "pt": pt,
            "ones16": ones16,
            "dmask": dmask,
        })
    return in_maps


def run(query_embeddings, positive_embeddings, trace=False):
    nc = _build()
    in_maps = _host_inputs(query_embeddings, positive_embeddings)
    res = run_bass_kernel_spmd(nc, in_maps, core_ids=list(range(N_CORES)),
                               trace=trace)
    total = 0.0
    for core in range(N_CORES):
        total += float(res.results[core]["loss_vec"].sum())
    loss = np.float32(total / B)
    return loss, res


def kernel(query_embeddings, positive_embeddings):
    loss, _ = run(query_embeddings, positive_embeddings)
    return loss


# revision 6
# speedup vs baseline: 1.0136x; 1.0136x over previous
"""ColBERT in-batch-negatives loss on 8 Trainium2 NeuronCores.

Sharding: batch (b) axis of query_embeddings split across the 8 cores
(16 rows each); every core receives the full positive_embeddings (the
"all-gather" is done at input-distribution time since kernel() takes the
full inputs anyway). Each core computes its [16, 128] score slab

    score[b, c] = sum_s max_d  q[b, s, :] . p[c, d, :]

via PE matmuls (bf16 inputs, fp32 PSUM) + DVE segmented max-reduce + a
ones-matmul for the sum over s, then the per-sample CE partial
    loss[b] = logsumexp_c(score[b, :] / T) - score[b, b] / T
on-device. The host sums the 8x16 per-sample losses and divides by 128
(the mean "all-reduce" at unshard time).

B=128, S=32, D_TOK=128, H=128, TEMPERATURE=0.02 are hardcoded per spec.
"""
import numpy as np

import concourse.mybir as mybir
from concourse import bacc
from concourse.tile import TileContext
from concourse.bass_utils import run_bass_kernel_spmd

F32 = mybir.dt.float32
BF16 = mybir.dt.bfloat16

B, S, D_TOK, H = 128, 32, 128, 128
TEMPERATURE = 0.02
N_CORES = 8
B_LOC = B // N_CORES            # 16 batch rows per core
N_BG = B_LOC // 4               # 4 b-groups of 4 rows (4*32 = 128 partitions)
CD = B * D_TOK                  # 16384 columns of p^T
CHUNK = 2048                    # psum tile free size (4 banks), 16 c's
N_CHUNK = CD // CHUNK           # 8 chunks

_cache = {}


def _build():
    """Build + compile the SPMD bass kernel (once per process)."""
    if "nc" in _cache:
        return _cache["nc"]

    nc = bacc.Bacc("TRN2", target_bir_lowering=False, debug=False,
                   num_devices=N_CORES)
    qt = nc.dram_tensor("qt", [H, B_LOC * S], BF16, kind="ExternalInput").ap()
    pt = nc.dram_tensor("pt", [H, CD], BF16, kind="ExternalInput").ap()
    ones16 = nc.dram_tensor("ones16", [H, 4 * B_LOC], F32,
                            kind="ExternalInput").ap()
    dmask = nc.dram_tensor("dmask", [B_LOC, B], F32, kind="ExternalInput").ap()
    loss_vec = nc.dram_tensor("loss_vec", [B_LOC, 1], F32,
                              kind="ExternalOutput").ap()

    with TileContext(nc) as tc:
        with tc.tile_pool(name="sbuf", bufs=1) as pool, \
             tc.tile_pool(name="psum", bufs=1, space="PSUM") as psum_pool:
            qt_t = pool.tile([H, B_LOC * S], BF16)
            ones_t = pool.tile([H, 4 * B_LOC], F32)
            dmask_t = pool.tile([B_LOC, B], F32)
            # separate tile per chunk so the first matmul only waits on the
            # first chunk's DMA, not the whole 4 MiB transfer
            pt_tiles = [pool.tile([H, CHUNK], BF16, name=f"ptc{_j}")
                        for _j in range(N_CHUNK)]
            with nc.named_scope("load"):
                nc.sync.dma_start(qt_t[:], qt[:])
                nc.sync.dma_start(ones_t[:], ones16[:])
                nc.sync.dma_start(dmask_t[:], dmask[:])
                for j in range(N_CHUNK):
                    nc.sync.dma_start(pt_tiles[j][:],
                                        pt[:, j * CHUNK:(j + 1) * CHUNK])

            pA = psum_pool.tile([128, CHUNK], F32, name="pA")
            pB = psum_pool.tile([128, CHUNK], F32, name="pB")
            ptiles = [pA, pB]

            # m_all[:, g*128 + c] = max_d late for b-group g, batch-col c
            m_all = pool.tile([128, 4 * B], F32)

            with nc.named_scope("mm_reduce"):
                for g in range(N_BG):
                    stat = qt_t[:, g * 128:(g + 1) * 128]
                    for j in range(N_CHUNK):
                        pt_tile = ptiles[(g * N_CHUNK + j) % 2]
                        for k in range(CHUNK // 512):
                            nc.tensor.matmul(
                                pt_tile[:, k * 512:(k + 1) * 512],
                                stat,
                                pt_tiles[j][:, k * 512:(k + 1) * 512],
                                start=True, stop=True)
                        # segmented max over d: [128, 16, 128] -> [128, 16]
                        nc.vector.tensor_reduce(
                            m_all[:, g * B + j * (CHUNK // D_TOK):
                                  g * B + (j + 1) * (CHUNK // D_TOK)],
                            pt_tile[:].rearrange("p (c d) -> p c d",
                                                 d=D_TOK),
                            axis=mybir.AxisListType.X,
                            op=mybir.AluOpType.max)

            # scores[b, c] = sum_s m_all: 4 accumulating ones-matmuls into
            # partitions 0..15 of pA's first bank
            s_psum = pA[0:B_LOC, 0:B]
            with nc.named_scope("tail"):
                for g in range(N_BG):
                    nc.tensor.matmul(
                        s_psum, ones_t[:, g * B_LOC:(g + 1) * B_LOC],
                        m_all[:, g * B:(g + 1) * B],
                        start=(g == 0), stop=(g == N_BG - 1))

                s_all = pool.tile([B_LOC, B], F32)
                nc.scalar.activation(s_all[:], s_psum,
                                     mybir.ActivationFunctionType.Copy,
                                     bias=0.0, scale=1.0 / TEMPERATURE)
                r = pool.tile([B_LOC, 1], F32)
                nc.vector.tensor_reduce(r[:], s_all[:],
                                        axis=mybir.AxisListType.X,
                                        op=mybir.AluOpType.max)
                negr = pool.tile([B_LOC, 1], F32)
                nc.vector.tensor_scalar_mul(negr[:], r[:], -1.0)
                e = pool.tile([B_LOC, B], F32)
                z = pool.tile([B_LOC, 1], F32)
                nc.scalar.activation(e[:], s_all[:],
                                     mybir.ActivationFunctionType.Exp,
                                     bias=negr[:], scale=1.0,
                                     accum_out=z[:])
                logz = pool.tile([B_LOC, 1], F32)
                nc.scalar.activation(logz[:], z[:],
                                     mybir.ActivationFunctionType.Ln)
                lse = pool.tile([B_LOC, 1], F32)
                nc.vector.tensor_tensor(lse[:], r[:], logz[:],
                                        op=mybir.AluOpType.add)
                junk = pool.tile([B_LOC, B], F32)
                diag = pool.tile([B_LOC, 1], F32)
                nc.vector.tensor_tensor(junk[:], s_all[:], dmask_t[:],
                                        op=mybir.AluOpType.mult)
                nc.vector.tensor_reduce(diag[:], junk[:],
                                        axis=mybir.AxisListType.X,
                                        op=mybir.AluOpType.add)
                lv = pool.tile([B_LOC, 1], F32)
                nc.vector.tensor_tensor(lv[:], lse[:], diag[:],
                                        op=mybir.AluOpType.subtract)
                nc.sync.dma_start(loss_vec[:], lv[:])

    nc.compile()
    _cache["nc"] = nc
    return nc


def _host_inputs(query_embeddings, positive_embeddings):
    """Shard + lay out host-side inputs for the 8 cores."""
    import ml_dtypes
    q = np.ascontiguousarray(query_embeddings, dtype=np.float32)
    p = np.ascontiguousarray(positive_embeddings, dtype=np.float32)
    # qt_full[h, b*S + s] = q[b, s, h]
    qt_full = np.ascontiguousarray(
        q.transpose(2, 0, 1).reshape(H, B * S)).astype(ml_dtypes.bfloat16)
    # pt[h, c*D + d] = p[c, d, h]
    pt = np.ascontiguousarray(
        p.transpose(2, 0, 1).reshape(H, CD)).astype(ml_dtypes.bfloat16)

    ones16 = np.zeros((H, 4 * B_LOC), dtype=np.float32)
    for g in range(N_BG):
        for k in range(128):
            ones16[k, g * B_LOC + g * 4 + k // S] = 1.0

    in_maps = []
    for core in range(N_CORES):
        dmask = np.zeros((B_LOC, B), dtype=np.float32)
        for i in range(B_LOC):
            dmask[i, core * B_LOC + i] = 1.0
        in_maps.append({
            "qt": np.ascontiguousarray(
                qt_full[:, core * B_LOC * S:(core + 1) * B_LOC * S]),
            "pt": pt,
            "ones16": ones16,
            "dmask": dmask,
        })
    return in_maps


def run(query_embeddings, positive_embeddings, trace=False):
    nc = _build()
    in_maps = _host_inputs(query_embeddings, positive_embeddings)
    res = run_bass_kernel_spmd(nc, in_maps, core_ids=list(range(N_CORES)),
                               trace=trace)
    total = 0.0
    for core in range(N_CORES):
        total += float(res.results[core]["loss_vec"].sum())
    loss = np.float32(total / B)
    return loss, res


def kernel(query_embeddings, positive_embeddings):
    loss, _ = run(query_embeddings, positive_embeddings)
    return loss
